# revision 1
# baseline (speedup 1.0000x reference)
"""ADBS loss kernel for 8 TRN2 NeuronCores.

total_loss = CE(logits, targets) + ALPHA * IC(prototypes, boundaries)

Sharding (data-parallel, no collectives):
  - logits/targets: batch-sharded across 8 cores (2048 rows each, bf16 compute).
  - Gram/IC: prototype rows sharded (512 per core); the transposed operands
    (P_local^T and (P*(b-1))^T, bf16) are replicated host-side layout prep.
  - Each core emits out[128, 2*NT+1] = (per-row exp-sums, per-row target
    logits, ic partial); the host combine does ln + the final scalar sums.

Per-core device graph (one Bass/Tile NEFF):
  CE:  16 tiles of [128, 4096] bf16. ACT: exp with accum_out gives per-row
       sum(exp(x)) in one pass (max-subtraction skipped: logits ~ N(0,1), no
       overflow). The target logit x[r, t_r] is gathered exactly via a
       per-tile GPSIMD indirect DMA with host-computed flat indices.
       nll_r = ln(sum_r) - x[r, t_r] (the ln runs in the host combine to
       keep a second ACT table-set load off the device critical tail).
  IC:  PE matmul lhsT=P_local^T x rhs=(P*(b-1))^T accumulates bm1_j*G_ij in
       PSUM (the (b_j-1) scale is folded into the replicated operand, kk-outer
       over 4 rotating PSUM banks). DVE: tensor_scalar relu(ps + term1_i) to
       bf16, then one sum-accumulate per 128-row block. term1=(1-b_i)*||p_i||^2
       comes precomputed. The diagonal contributes ~0 by construction.

Scheduling (the part that matters for perf):
  - ALL big DMAs ride the sync (SP) HWDGE ring in hand-interleaved FIFO order:
    pt chunks woven between the first logits tiles, so PE starts at ~10us and
    the exp stream is never starved. ACT issues no DMAs (its sequencer time is
    the CE critical path: 16 x ~4.0us exp cadence).
  - gpsimd SWDGE: 16 tiny indirect gathers, fully overlapped.
  - Dual-ring layouts and device-side xbar transposes measured slower
    (xbar transpose from strided DRAM ~35 GB/s; dual-ring logits collapses
    per-ring throughput and adds run-to-run variance).
"""

import numpy as np
import ml_dtypes

B, C, D = 16384, 4096, 768
NCORES = 8
BL = B // NCORES       # 2048 logit rows per core
RL = C // NCORES       # 512 prototype rows per core
ALPHA = 0.05
NT = BL // 128         # 16 CE tiles
MC = RL // 128         # 4 gram row-chunks
NNC = C // 512         # 8 gram col-chunks
KC = D // 128          # 6 contraction chunks

_CACHE = {}


def _build_nc():
    from concourse import bacc
    import concourse.bass as bass
    import concourse.mybir as mybir
    import concourse.tile as tile

    f32 = mybir.dt.float32
    bf16 = mybir.dt.bfloat16
    i32 = mybir.dt.int32
    AF = mybir.ActivationFunctionType
    OP = mybir.AluOpType

    nc = bacc.Bacc(
        "TRN2", target_bir_lowering=False, debug=False, num_devices=NCORES
    )

    logits_d = nc.dram_tensor("logits", [BL, C], bf16, kind="ExternalInput")
    idx_d = nc.dram_tensor("idx", [128, NT], i32, kind="ExternalInput")
    ptb_d = nc.dram_tensor("ptb", [D, C], bf16, kind="ExternalInput")
    ptlb_d = nc.dram_tensor("ptlb", [D, RL], bf16, kind="ExternalInput")
    t1_d = nc.dram_tensor("t1", [128, MC], f32, kind="ExternalInput")
    out_d = nc.dram_tensor("out", [128, 2 * NT + 3], f32, kind="ExternalOutput")

    logits_flat = logits_d[:].rearrange("a (b o) -> (a b) o", o=1)

    with tile.TileContext(nc) as tc:
        with (
            tc.tile_pool(name="const", bufs=1) as cpool,
            tc.tile_pool(name="stream", bufs=8) as spool,
            tc.tile_pool(name="ic", bufs=2) as icpool,
            tc.tile_pool(name="psum", bufs=2, space=bass.MemorySpace.PSUM) as ppool,
        ):
            # -------- setup --------
            # Sync-ring FIFO order: first two logits tiles absolutely first
            # (exp0 right after the ACT preamble), then small inputs and the
            # first pt chunks woven in; pt chunks 3-5 ride the scalar ring
            # (fair 1:1 packet split since logits tiles are bf16 too).
            idx_sb = cpool.tile([128, NT], i32)
            term1 = cpool.tile([128, MC], f32)        # (1 - b_i) * ||p_i||^2
            ptl = cpool.tile([128, KC, RL], bf16)     # P_local^T
            pt = cpool.tile([128, KC, C], bf16)       # (P*(b-1))^T chunks
            xt_tiles = []

            def _xt_dma(t):
                xt = spool.tile([128, C], bf16, tag="xt")
                nc.sync.dma_start(xt[:], logits_d[128 * t:128 * (t + 1), :])
                xt_tiles.append(xt)

            _xt_dma(0)
            nc.sync.dma_start(idx_sb[:], idx_d[:])
            _xt_dma(1)
            nc.sync.dma_start(term1[:], t1_d[:])
            _xt_dma(2)
            nc.sync.dma_start(
                ptl[:], ptlb_d[:].rearrange("(k p) r -> p k r", p=128)
            )
            nc.sync.dma_start(pt[:, 0, :], ptb_d[0:128, :])
            for kk in range(3, KC):
                nc.scalar.dma_start(
                    pt[:, kk, :], ptb_d[128 * kk:128 * (kk + 1), :]
                )
            for t in range(3, 6):
                _xt_dma(t)
                if t - 2 <= 2:
                    nc.sync.dma_start(
                        pt[:, t - 2, :], ptb_d[128 * (t - 2):128 * (t - 1), :]
                    )

            ex = cpool.tile([128, C], bf16)           # exp trash output
            sums = cpool.tile([128, NT], f32)
            sumsx = cpool.tile([128, 2], f32)         # B-half sums, tiles 0-1
            picked = cpool.tile([128, NT], bf16)
            icp = cpool.tile([128, MC], f32)

            # ---------------- CE ----------------
            for t in range(NT):
                if t < len(xt_tiles):
                    xt = xt_tiles[t]
                else:
                    xt = spool.tile([128, C], bf16, tag="xt")
                    nc.sync.dma_start(xt[:], logits_d[128 * t:128 * (t + 1), :])
                nc.gpsimd.indirect_dma_start(
                    out=picked[:, t:t + 1],
                    out_offset=None,
                    in_=logits_flat,
                    in_offset=bass.IndirectOffsetOnAxis(
                        ap=idx_sb[:, t:t + 1], axis=0
                    ),
                )
                if t < 2:
                    nc.scalar.activation(
                        ex[:, 0:C // 2], xt[:, 0:C // 2], AF.Exp,
                        accum_out=sums[:, t:t + 1]
                    )
                    nc.scalar.activation(
                        ex[:, C // 2:C], xt[:, C // 2:C], AF.Exp,
                        accum_out=sumsx[:, t:t + 1]
                    )
                else:
                    nc.scalar.activation(
                        ex[:], xt[:], AF.Exp, accum_out=sums[:, t:t + 1]
                    )

            # ---------------- IC ----------------
            # kk-outer over 4 rotating PSUM banks: consecutive matmuls hit
            # different banks and reuse the same stationary lhsT.
            for m in range(MC):
                r = icpool.tile([128, NNC, 512], bf16, tag="r")
                for h in range(2):
                    ps = ppool.tile([128, 4, 512], f32, tag="ps")
                    for kk in range(KC):
                        for nn in range(4):
                            n = 4 * h + nn
                            nc.tensor.matmul(
                                ps[:, nn, :],
                                ptl[:, kk, 128 * m:128 * (m + 1)],
                                pt[:, kk, 512 * n:512 * (n + 1)],
                                start=(kk == 0),
                                stop=(kk == KC - 1),
                            )
                    for nn in range(4):
                        n = 4 * h + nn
                        # r[:, n] = relu(ps_nn + term1_m) -> bf16
                        nc.vector.tensor_scalar(
                            out=r[:, n, :], in0=ps[:, nn, :],
                            scalar1=term1[:, m:m + 1],
                            scalar2=0.0, op0=OP.add, op1=OP.max,
                        )
                # icp[m] = sum over the whole m-chunk row block
                nc.vector.tensor_scalar(
                    out=r[:], in0=r[:], scalar1=0.0,
                    scalar2=None, op0=OP.add, op1=OP.add,
                    accum_out=icp[:, m:m + 1],
                )

            # ---------------- finalize ----------------
            # Ship per-row sums/picked + the ic partial; ln and the final
            # reduction happen in the host combine (avoids a second ACT
            # table-set load + drain on the critical tail).
            outsb = cpool.tile([128, 2 * NT + 3], f32)
            nc.vector.tensor_copy(outsb[:, 0:NT], sums[:])
            nc.vector.tensor_copy(outsb[:, NT:2 * NT], picked[:])
            nc.vector.tensor_reduce(
                out=outsb[:, 2 * NT:2 * NT + 1], in_=icp[:],
                axis=mybir.AxisListType.X, op=OP.add,
            )
            nc.vector.tensor_copy(outsb[:, 2 * NT + 1:2 * NT + 3], sumsx[:])
            nc.sync.dma_start(out_d[:], outsb[:])

    nc.compile()
    return nc


def _get_nc():
    if "nc" not in _CACHE:
        _CACHE["nc"] = _build_nc()
    return _CACHE["nc"]


def _make_in_maps(logits, targets, prototypes, boundaries):
    logits = np.asarray(logits)
    targets = np.asarray(targets)
    prototypes = np.asarray(prototypes)
    boundaries = np.asarray(boundaries)

    assert logits.shape == (B, C) and prototypes.shape == (C, D)
    logits = logits.astype(ml_dtypes.bfloat16)
    tgt = targets.astype(np.int64).reshape(NCORES, NT, 128)
    rows = np.arange(BL).reshape(NT, 128)
    bnd = boundaries.astype(np.float32)
    prot = np.asarray(prototypes, dtype=np.float32)
    pbs = (prot * (bnd - 1.0)[:, None]).astype(ml_dtypes.bfloat16)
    ptb = np.ascontiguousarray(pbs.T)                 # [D, C]
    pbf_t = np.ascontiguousarray(prot.astype(ml_dtypes.bfloat16).T)
    d2 = (prot.astype(np.float64) ** 2).sum(1).astype(np.float32)  # ||p_i||^2
    t1_full = (1.0 - bnd) * d2                        # (1-b_i) * d_i
    in_maps = []
    for k in range(NCORES):
        # idx[p, t] = flat index of (row 128t+p, targets[row]) in the core's shard
        idx = (rows * C + tgt[k]).astype(np.int32).T  # [128, NT]
        t1 = np.ascontiguousarray(
            t1_full[k * RL:(k + 1) * RL].reshape(MC, 128).T
        )
        in_maps.append({
            "logits": logits[k * BL:(k + 1) * BL],
            "idx": np.ascontiguousarray(idx),
            "ptb": ptb,
            "ptlb": np.ascontiguousarray(pbf_t[:, k * RL:(k + 1) * RL]),
            "t1": t1,
        })
    return in_maps


def _combine(results):
    outs = np.stack([np.asarray(r["out"]) for r in results])  # [8, 128, 2*NT+1]
    sums = outs[:, :, 0:NT].astype(np.float64).copy()
    sums[:, :, 0:2] += outs[:, :, 2 * NT + 1:2 * NT + 3].astype(np.float64)
    picked = outs[:, :, NT:2 * NT].astype(np.float64)
    nll_sum = (np.log(sums) - picked).sum()
    ic_sum = outs[:, :, 2 * NT].astype(np.float64).sum()
    cls = nll_sum / B
    ic = ic_sum / (C * (C - 1))
    total = cls + ALPHA * ic
    return (np.float32(total), np.float32(cls), np.float32(ic))


def kernel(logits, targets, prototypes, boundaries, _trace=False):
    from concourse.bass_utils import run_bass_kernel_spmd

    nc = _get_nc()
    in_maps = _make_in_maps(logits, targets, prototypes, boundaries)
    res = run_bass_kernel_spmd(
        nc, in_maps, core_ids=list(range(NCORES)), trace=_trace
    )
    out = _combine(res.results)
    if _trace:
        _CACHE["last_result"] = res
    return out



# revision 3
# speedup vs baseline: 1.1004x; 1.1004x over previous
"""ADBS loss kernel for 8 TRN2 NeuronCores.

total_loss = CE(logits, targets) + ALPHA * IC(prototypes, boundaries)

Sharding (data-parallel, no collectives):
  - logits/targets: batch-sharded across 8 cores (2048 rows each, fp8 e4m3
    compute; exp error from quantization is ~6e-6 rel on cls).
  - Gram/IC: prototype rows sharded (512 per core); transposed operands fp8.

Per-core device graph (one Bass/Tile NEFF):
  CE:  16 tiles of [128, 4096] fp8, split 3200/896 between ACT and DVE:
       - ACT: exp over cols [0:3200] with accum_out -> per-row partial sum.
         ACT is the 1 elem/cycle/lane bottleneck engine; offloading 896
         cols to DVE shaves ~12us off its critical path.
       - DVE: custom-op exp over cols [3200:4096]:
           EXP_POLY_ANT:      p = 0.5*(x/128 + 1)^2 + 0.5   (= 1+t+t^2/2)
           EXP_SQ_REDUCE_ANT: p^128 via 7 squarings, accum_out = row sum.
         exp(x) ~= (1+t+t^2/2)^128, t=x/128: rel err <= x^3/(6*2^14),
         max 0.1% at |x|<=5.5, Z-weighted bias ~ -4e-5. No bit tricks.
       Target logits gathered via per-tile GPSIMD indirect DMA (fp8 bytes).
  IC:  PE fp8 DoubleRow matmuls (K=256/instr, 2x): lhsT=P_local^T x
       rhs=(P*(b-1))^T, 3 kk-pairs x 4 PSUM banks per half-chunk, bufs=2.
       DVE custom op RELU_ACC_ANT: relu(ps + term1_i) over [128, 4, 512]
       with accum=add in ONE instruction (tensor_scalar can't: its accum
       reduction op is op1, which relu needs for max).

Scheduling:
  - Scalar queue: table load + 16 ACTIVATE + 16 READ_ACC only (~53us).
  - Sync ring: all DMAs, fp8 halves the bytes (11.4MB ~= 32us), ordered so
    ACT never starves and pt lands by ~19us (PE then runs 19-42us).
  - Vector queue: per-tile exp pairs interleaved with IC relu drains.
"""

import numpy as np
import ml_dtypes
from operator import add as _add

B, C, D = 16384, 4096, 768
NCORES = 8
BL = B // NCORES       # 2048 logit rows per core
RL = C // NCORES       # 512 prototype rows per core
ALPHA = 0.05
NT = BL // 128         # 16 CE tiles
MC = RL // 128         # 4 gram row-chunks
KC = D // 128          # 6 contraction chunks
KP = KC // 2           # 3 DoubleRow pair chunks
XC = 896               # CE columns offloaded to DVE per tile
CA = C - XC            # CE columns on ACT per tile

_CACHE = {}


def _register_dve_ops():
    """Register the custom DVE ops via the documented extension point
    (dve_ops.OPS); rows 17+ are free on trn2 (row field allows [1, 0x20))."""
    if "ops" in _CACHE:
        return _CACHE["ops"]
    from concourse import dve_ops
    from concourse.dve_spec import Spec, Src0, C0, C1, C2, lower, relu, sq
    from concourse.dve_uop import DveOpSpec

    def _reg(name, spec):
        for o in dve_ops.OPS:
            if o.name == name:
                return o
        row = dve_ops._CUSTOM_DVE_ROW_BASE + len(dve_ops.OPS)
        assert row < 0x20
        dve_ops._SUB_OPCODE_FOR_NAME[name] = row
        shas = {}
        for ver in ("v3", "v4"):
            u = lower(spec, ver=ver)
            shas[ver] = DveOpSpec(name=name, opcode=row, uops=u, rd1_en=False).sha(ver)
        op = dve_ops.DveOp(name=name, spec=spec, subdim=False, uops_sha=shas)
        dve_ops.OPS.append(op)
        dve_ops.CUSTOM_DVE_SPECS[name] = spec
        return op

    def _relu_ref(in0, in1, c0, c1, c2):
        b = np.maximum(
            np.nan_to_num(in0.astype(np.float32) + c0, nan=0.0), 0
        ).astype(np.float32)
        return b, c1 + b.reshape(b.shape[0], -1).sum(-1, keepdims=True).astype(
            np.float32
        )

    def _poly_ref(in0, in1, c0, c1, c2):
        u = in0.astype(np.float32) * c0 + c1
        return (u * u * c2 + c2).astype(np.float32)

    def _sq_ref(in0, in1, c0, c1, c2):
        v = in0.astype(np.float32)
        for _ in range(7):
            v = v * v
        return v, c0 + v.reshape(v.shape[0], -1).sum(-1, keepdims=True).astype(
            np.float32
        )

    relu_acc = _reg(
        "RELU_ACC_ANT",
        Spec(body=relu(Src0 + C0), accum=_add, accum_init=C1, reference=_relu_ref),
    )
    exp_poly = _reg(
        "EXP_POLY_ANT",
        Spec(body=sq(Src0 * C0 + C1) * C2 + C2, reference=_poly_ref),
    )
    b = Src0
    for _ in range(7):
        b = sq(b)
    exp_sq = _reg(
        "EXP_SQ_REDUCE_ANT",
        Spec(body=b, accum=_add, accum_init=C0, reference=_sq_ref),
    )
    _CACHE["ops"] = (relu_acc, exp_poly, exp_sq)
    return _CACHE["ops"]


def _build_nc():
    from concourse import bacc
    import concourse.bass as bass
    import concourse.mybir as mybir
    import concourse.tile as tile

    RELU_ACC, EXP_POLY, EXP_SQ = _register_dve_ops()

    f32 = mybir.dt.float32
    bf16 = mybir.dt.bfloat16
    fp8 = mybir.dt.float8e4
    i32 = mybir.dt.int32
    AF = mybir.ActivationFunctionType
    OP = mybir.AluOpType
    DR = mybir.MatmulPerfMode.DoubleRow

    nc = bacc.Bacc(
        "TRN2", target_bir_lowering=False, debug=False, num_devices=NCORES
    )

    logits_d = nc.dram_tensor("logits", [BL, C], fp8, kind="ExternalInput")
    idx_d = nc.dram_tensor("idx", [128, NT], i32, kind="ExternalInput")
    ptb_d = nc.dram_tensor("ptb", [D, C], fp8, kind="ExternalInput")
    ptlb_d = nc.dram_tensor("ptlb", [D, RL], fp8, kind="ExternalInput")
    t1_d = nc.dram_tensor("t1", [128, MC], f32, kind="ExternalInput")
    out_d = nc.dram_tensor("out", [128, 3 * NT + 1], f32, kind="ExternalOutput")

    logits_flat = logits_d[:].rearrange("a (b o) -> (a b) o", o=1)

    with tile.TileContext(nc) as tc:
        with (
            tc.tile_pool(name="const", bufs=1) as cpool,
            tc.tile_pool(name="stream", bufs=8) as spool,
            tc.tile_pool(name="ic", bufs=2) as icpool,
            tc.tile_pool(name="dve", bufs=2) as dpool,
            tc.tile_pool(name="psum", bufs=2, space=bass.MemorySpace.PSUM) as ppool,
        ):
            # -------- setup: sync-ring FIFO order --------
            idx_sb = cpool.tile([128, NT], i32)
            term1 = cpool.tile([128, MC], f32)        # (1 - b_i) * ||p_i||^2
            ptl = cpool.tile([128, KC, RL], fp8)      # P_local^T
            pt = cpool.tile([128, KC, C], fp8)        # (P*(b-1))^T
            xt_tiles = []

            def _xt_dma(t):
                xt = spool.tile([128, C], fp8, tag="xt")
                nc.sync.dma_start(xt[:], logits_d[128 * t:128 * (t + 1), :])
                xt_tiles.append(xt)

            def _pt_dma(kk):
                nc.sync.dma_start(
                    pt[:, 2 * kk:2 * kk + 2, :],
                    ptb_d[256 * kk:256 * (kk + 1), :].rearrange(
                        "(k p) c -> p k c", p=128
                    ),
                )

            _xt_dma(0)
            nc.sync.dma_start(idx_sb[:], idx_d[:])
            nc.sync.dma_start(term1[:], t1_d[:])
            _xt_dma(1)
            nc.sync.dma_start(
                ptl[:], ptlb_d[:].rearrange("(k p) r -> p k r", p=128)
            )
            _xt_dma(2)
            _xt_dma(3)
            _pt_dma(0)
            _xt_dma(4)
            _pt_dma(1)
            _xt_dma(5)
            _pt_dma(2)

            ex = cpool.tile([128, CA], bf16)          # exp trash output
            picked = cpool.tile([128, NT], fp8)
            icp = cpool.tile([128, 2 * MC], f32)
            outsb = cpool.tile([128, 3 * NT + 1], f32)

            def _ic_group(g):
                m, h = g // 2, g % 2
                ps = ppool.tile([128, 4, 512], f32, tag="ps")
                for kk in range(KP):
                    for nn in range(4):
                        n = 4 * h + nn
                        nc.tensor.matmul(
                            ps[:, nn, :],
                            ptl[:, 2 * kk:2 * kk + 2, 128 * m:128 * (m + 1)],
                            pt[:, 2 * kk:2 * kk + 2, 512 * n:512 * (n + 1)],
                            start=(kk == 0),
                            stop=(kk == KP - 1),
                            perf_mode=DR,
                        )
                rt = icpool.tile([128, 4, 512], bf16, tag="rt")
                nc.vector._custom_dve(
                    RELU_ACC, out=rt[:], in0=ps[:],
                    s0=term1[:, m:m + 1], s1=0.0,
                    accum_out=icp[:, g:g + 1],
                )

            # ---------------- CE (+ interleaved IC) ----------------
            for t in range(NT):
                if t < len(xt_tiles):
                    xt = xt_tiles[t]
                else:
                    xt = spool.tile([128, C], fp8, tag="xt")
                    nc.sync.dma_start(xt[:], logits_d[128 * t:128 * (t + 1), :])
                nc.gpsimd.indirect_dma_start(
                    out=picked[:, t:t + 1],
                    out_offset=None,
                    in_=logits_flat,
                    in_offset=bass.IndirectOffsetOnAxis(
                        ap=idx_sb[:, t:t + 1], axis=0
                    ),
                )
                nc.scalar.activation(
                    ex[:], xt[:, 0:CA], AF.Exp, accum_out=outsb[:, t:t + 1]
                )
                if XC:
                    p = dpool.tile([128, XC], f32, tag="p")
                    nc.vector._custom_dve(
                        EXP_POLY, out=p[:], in0=xt[:, CA:C],
                        s0=1.0 / 128.0, s1=1.0, imm2=0.5,
                    )
                    zt = dpool.tile([128, XC], bf16, tag="zt")
                    nc.vector._custom_dve(
                        EXP_SQ, out=zt[:], in0=p[:], s0=0.0, s1=0.0,
                        accum_out=outsb[:, 2 * NT + t:2 * NT + t + 1],
                    )
                if 7 <= t < 7 + 8:
                    _ic_group(t - 7)

            # ---------------- finalize ----------------
            nc.vector.tensor_copy(outsb[:, NT:2 * NT], picked[:])
            nc.vector.tensor_reduce(
                out=outsb[:, 3 * NT:3 * NT + 1], in_=icp[:],
                axis=mybir.AxisListType.X, op=OP.add,
            )
            nc.sync.dma_start(out_d[:], outsb[:])

    nc.compile()
    return nc


def _get_nc():
    if "nc" not in _CACHE:
        _CACHE["nc"] = _build_nc()
    return _CACHE["nc"]


def _make_in_maps(logits, targets, prototypes, boundaries):
    logits = np.asarray(logits)
    targets = np.asarray(targets)
    prototypes = np.asarray(prototypes)
    boundaries = np.asarray(boundaries)

    assert logits.shape == (B, C) and prototypes.shape == (C, D)
    logits = logits.astype(ml_dtypes.float8_e4m3)
    tgt = targets.astype(np.int64).reshape(NCORES, NT, 128)
    rows = np.arange(BL).reshape(NT, 128)
    bnd = boundaries.astype(np.float32)
    prot = np.asarray(prototypes, dtype=np.float32)
    pbs = (prot * (bnd - 1.0)[:, None]).astype(ml_dtypes.float8_e4m3)
    ptb = np.ascontiguousarray(pbs.T)                 # [D, C]
    pbf_t = np.ascontiguousarray(prot.astype(ml_dtypes.float8_e4m3).T)
    d2 = (prot.astype(np.float64) ** 2).sum(1).astype(np.float32)  # ||p_i||^2
    t1_full = (1.0 - bnd) * d2                        # (1-b_i) * d_i
    in_maps = []
    for k in range(NCORES):
        # idx[p, t] = flat index of (row 128t+p, targets[row]) in the core's shard
        idx = (rows * C + tgt[k]).astype(np.int32).T  # [128, NT]
        t1 = np.ascontiguousarray(
            t1_full[k * RL:(k + 1) * RL].reshape(MC, 128).T
        )
        in_maps.append({
            "logits": logits[k * BL:(k + 1) * BL],
            "idx": np.ascontiguousarray(idx),
            "ptb": ptb,
            "ptlb": np.ascontiguousarray(pbf_t[:, k * RL:(k + 1) * RL]),
            "t1": t1,
        })
    return in_maps


def _combine(results):
    outs = np.stack([np.asarray(r["out"]) for r in results])  # [8, 128, 3*NT+1]
    sums = outs[:, :, 0:NT].astype(np.float64)
    if XC:
        sums = sums + outs[:, :, 2 * NT:3 * NT].astype(np.float64)
    picked = outs[:, :, NT:2 * NT].astype(np.float64)
    nll_sum = (np.log(sums) - picked).sum()
    ic_sum = outs[:, :, 3 * NT].astype(np.float64).sum()
    cls = nll_sum / B
    ic = ic_sum / (C * (C - 1))
    total = cls + ALPHA * ic
    return (np.float32(total), np.float32(cls), np.float32(ic))


def kernel(logits, targets, prototypes, boundaries, _trace=False):
    from concourse.bass_utils import run_bass_kernel_spmd

    nc = _get_nc()
    in_maps = _make_in_maps(logits, targets, prototypes, boundaries)
    res = run_bass_kernel_spmd(
        nc, in_maps, core_ids=list(range(NCORES)), trace=_trace
    )
    out = _combine(res.results)
    if _trace:
        _CACHE["last_result"] = res
    return out
